# revision 48
# baseline (speedup 1.0000x reference)
"""BatchHardTripletLoss on 8 Trainium2 NeuronCores (Bass/Tile).

Math: for the n x n squared-distance matrix d2[i,j] = sq_i + sq_j - 2*f_i.f_j,
hardest positive = max_{id_j==id_i} dist, hardest negative = min_{id_j!=id_i} dist,
loss = mean(relu(margin + pos - neg)).  Both extremes commute with sqrt/+sq_i,
so each core reduces P[i,j] = delta_j - 2*G[i,j] + BIG*same[i,j] where
delta_j = sq_j - S0, then adds sq_i + S0 back in a tiny epilogue.
The BIG*same and delta_j terms ride a single K=128 one-hot matmul accumulated
on top of the Gram matmul, so no elementwise masking pass is ever needed.

Sharding: rows are sorted by identity on the host; core k owns sorted rows
[k*1024,(k+1)*1024).  Each core receives the full feature matrix rotated so its
own rows sit at local columns [256,1280) - identity groups are contiguous after
the sort, so the hardest-positive row-max only needs a 512-wide window around
the diagonal instead of a second full-matrix pass.
"""

import numpy as np

N = 8192
D = 128
NCORES = 8
RPC = N // NCORES  # rows per core
RB = RPC // 128  # row blocks per core
CHUNK = 2048  # psum chunk (4 banks)
NCHUNK = N // CHUNK
ROW0 = 256  # local column offset of a core's own rows
BIG = 4096.0
S0 = 128.0
MARGIN = 0.2
NID = 64

_cache = {}


def _build_nc(wide_window):
    from contextlib import ExitStack

    import concourse.bass as bass
    import concourse.bacc as bacc
    import concourse.mybir as mybir
    import concourse.tile as tile
    from concourse.masks import make_identity

    f32 = mybir.dt.float32
    bf16 = mybir.dt.bfloat16
    i32 = mybir.dt.int32
    AX = mybir.AxisListType.X
    Alu = mybir.AluOpType
    Act = mybir.ActivationFunctionType

    nc = bacc.Bacc(trn_type="TRN2", target_bir_lowering=False, debug=False)
    # host pre-tiles features to [p, t, d] so each partition's data is one
    # contiguous 32KB DRAM run (row-major [N, D] would DMA as a 512B scatter)
    fcols = nc.dram_tensor("fcols", [128, N // 128, D], f32, kind="ExternalInput")
    deltarow = nc.dram_tensor("deltarow", [N], f32, kind="Internal")
    idcols = nc.dram_tensor("idcols", [N], i32, kind="ExternalInput")
    partial = nc.dram_tensor("partial", [1, 1], f32, kind="ExternalOutput")

    with ExitStack() as ctx:
        tc = ctx.enter_context(tile.TileContext(nc))
        singles = ctx.enter_context(tc.tile_pool(name="singles", bufs=1))
        sqp = ctx.enter_context(tc.tile_pool(name="sqp", bufs=2))
        psum = ctx.enter_context(tc.tile_pool(name="psum", bufs=2, space="PSUM"))

        ident = singles.tile([128, 128], f32)
        make_identity(nc, ident)
        iota_i = singles.tile([64, 1], i32)
        nc.gpsimd.iota(iota_i, pattern=[[0, 1]], base=0, channel_multiplier=1)
        iota_f = singles.tile([64, 1], f32)
        nc.gpsimd.tensor_copy(iota_f, iota_i)

        # ---- DMAs up front, interleaved across the two HW DGE queues so
        # round-0 data (features + ids) lands first ----
        fnat = singles.tile([128, N // 128, D], f32)
        idb_i = singles.tile([64, N], i32)
        ic = idcols.ap()
        for r in range(4):
            eng = nc.sync if r % 2 == 0 else nc.scalar
            other = nc.scalar if r % 2 == 0 else nc.sync
            eng.dma_start(
                fnat[:, r * 16 : (r + 1) * 16, :],
                fcols.ap()[:, r * 16 : (r + 1) * 16, :],
            )
            other.dma_start(
                idb_i[:, r * CHUNK : (r + 1) * CHUNK],
                bass.AP(
                    tensor=ic.tensor, offset=r * CHUNK, ap=[[0, 64], [1, CHUNK]]
                ),
            )

        # ---- per-round: sq chain, F^T transpose, delta chunk, X chunk ----
        sqnat = singles.tile([128, N // 128], f32)  # sq of row (t*128+p) at [p, t]
        ftb = singles.tile([128, N], bf16)
        sqT = singles.tile([16, 4, 128], f32)  # [chunk-partition, round, col]
        dbc = singles.tile([64, N], f32)
        X = singles.tile([128, N], bf16)
        dr = deltarow.ap()
        # phase A: sq chain + delta broadcast + F^T transposes.  All DVE
        # reduces are emitted before any X op so the DVE stream cannot
        # head-of-line block on the id-broadcast DMA.
        sq_reduces = []
        for r in range(4):
            cols = slice(r * CHUNK, (r + 1) * CHUNK)
            # sq of this round's rows (ACT square + DVE reduce)
            sqsc = sqp.tile([128, 16, D], bf16, tag="sqsc")
            nc.scalar.activation(sqsc, fnat[:, r * 16 : (r + 1) * 16, :], Act.Square)
            sq_reduces.append(
                nc.vector.tensor_reduce(
                    sqnat[:, r * 16 : (r + 1) * 16], sqsc, axis=AX, op=Alu.add
                )
            )
            # delta chunk: transpose sq -> row layout -> DRAM -> broadcast
            tq = psum.tile([128, CHUNK], f32, tag="big")
            nc.tensor.transpose(
                tq[0:16, 0:128], sqnat[:, r * 16 : (r + 1) * 16], ident
            )
            nc.scalar.activation(sqT[:, r, :], tq[0:16, 0:128], Act.Copy, bias=-S0)
            nc.scalar.dma_start(
                bass.AP(
                    tensor=dr.tensor, offset=r * CHUNK, ap=[[128, 16], [1, 128]]
                ),
                sqT[:, r, :],
            )
            nc.scalar.dma_start(
                dbc[:, cols],
                bass.AP(
                    tensor=dr.tensor, offset=r * CHUNK, ap=[[0, 64], [1, CHUNK]]
                ),
            )
            # F^T chunk in bf16 via PE transpose + ACT copyback
            tp = psum.tile([128, CHUNK], f32, tag="big")
            for i in range(16):
                nc.tensor.transpose(
                    tp[:, i * 128 : (i + 1) * 128], fnat[:, r * 16 + i, :], ident
                )
            nc.scalar.copy(ftb[:, cols], tp)
            if r == 0:
                ftm2 = singles.tile([128, RPC], bf16)  # -2 * own-rows slice
                nc.vector.tensor_scalar_mul(ftm2, ftb[:, ROW0 : ROW0 + RPC], -2.0)
        # phase B: X construction per chunk.  Explicit same-engine deps pin
        # every X op after the sq reduces in the DVE stream (the scheduler
        # otherwise reorders them ahead and head-of-line blocks on DMA).
        for r in range(4):
            cols = slice(r * CHUNK, (r + 1) * CHUNK)
            # X rhs rows 0:64 -> onehot(id_j)*delta_j; rows 64:128 -> 64*onehot
            i1 = nc.vector.scalar_tensor_tensor(
                X[0:64, cols],
                idb_i[:, cols],
                iota_f,
                dbc[:, cols],
                op0=Alu.is_equal,
                op1=Alu.mult,
            )
            i2 = nc.vector.tensor_scalar(
                X[64:128, cols],
                idb_i[:, cols],
                iota_f,
                64.0,
                op0=Alu.is_equal,
                op1=Alu.mult,
            )
            for xi in (i1, i2):
                tile.add_dep_helper(
                    xi.ins, sq_reduces[-1].ins, sync=False, reason="dve order"
                )
            if r == 0:
                # lhsT for extras: ones on top, 64*onehot(id_m) below
                XL = singles.tile([128, RPC], bf16)
                nc.vector.memset(XL[0:64, :], 1.0)
                nc.vector.tensor_copy(XL[64:128, :], X[64:128, ROW0 : ROW0 + RPC])

        # per-row-block epilogue biases: sq_m + S0 (and -BIG for the pos side)
        biasP = singles.tile([128, RB], f32)
        nc.vector.tensor_scalar_add(biasP, sqnat[:, 2 : 2 + RB], S0 - BIG)
        biasN = singles.tile([128, RB], f32)
        nc.vector.tensor_scalar_add(biasN, sqnat[:, 2 : 2 + RB], S0)

        # ---- main loop ----
        # MCH=1024 psum tiles x 4 bufs keep PE streaming (2 chunks in
        # flight ahead of the DVE reduces) so the HAM clock stays warm.
        MCH = 1024
        NMC = N // MCH
        negacc = singles.tile([128, RB, NMC], f32)
        posacc = singles.tile([128, RB, 2], f32)
        nc.vector.memset(posacc, -1e9)
        MMF = 512  # psum-bank-limited moving-operand width
        # c outer / rb inner: chunk-c work only needs X columns of chunk c,
        # so the main loop starts as soon as the first X chunk is ready
        for c in range(NMC):
            for rb in range(RB):
                if wide_window:
                    wlo, whi = 0, 2048
                else:
                    wlo, whi = rb * 128 + 64, rb * 128 + 576
                P = psum.tile([128, MCH], f32, tag="big")
                # grouped by stationary operand so LDWEIGHTS amortizes
                for s in range(MCH // MMF):
                    col = c * MCH + s * MMF
                    nc.tensor.matmul(
                        P[:, s * MMF : (s + 1) * MMF],
                        ftm2[:, rb * 128 : (rb + 1) * 128],
                        ftb[:, col : col + MMF],
                        start=True,
                        stop=False,
                    )
                for s in range(MCH // MMF):
                    col = c * MCH + s * MMF
                    nc.tensor.matmul(
                        P[:, s * MMF : (s + 1) * MMF],
                        XL[:, rb * 128 : (rb + 1) * 128],
                        X[:, col : col + MMF],
                        start=False,
                        stop=True,
                    )
                nc.vector.tensor_reduce(
                    negacc[:, rb, c : c + 1], P, axis=AX, op=Alu.min
                )
                lo = max(wlo, c * MCH) - c * MCH
                hi = min(whi, (c + 1) * MCH) - c * MCH
                if lo < hi:
                    nc.vector.tensor_reduce(
                        posacc[:, rb, c % 2 : c % 2 + 1],
                        P[:, lo:hi],
                        axis=AX,
                        op=Alu.max,
                    )

        # ---- epilogue: sqrt both sides, relu(margin + pos - neg), sum ----
        negmin = singles.tile([128, RB], f32)
        nc.vector.tensor_reduce(negmin, negacc, axis=AX, op=Alu.min)
        posmax = singles.tile([128, RB], f32)
        nc.vector.tensor_reduce(posmax, posacc, axis=AX, op=Alu.max)
        posd2 = singles.tile([128, RB], f32)
        nc.vector.tensor_tensor(posd2, posmax, biasP, op=Alu.add)
        negd2 = singles.tile([128, RB], f32)
        nc.vector.tensor_tensor(negd2, negmin, biasN, op=Alu.add)
        posd = singles.tile([128, RB], f32)
        nc.scalar.activation(posd, posd2, Act.Sqrt)
        negd = singles.tile([128, RB], f32)
        nc.scalar.activation(negd, negd2, Act.Sqrt)
        term = singles.tile([128, RB], f32)
        nc.vector.scalar_tensor_tensor(
            term, posd, MARGIN, negd, op0=Alu.add, op1=Alu.subtract
        )
        termr = singles.tile([128, RB], f32)
        nc.vector.tensor_scalar_max(termr, term, 0.0)
        termsum = singles.tile([128, 1], f32)
        nc.vector.tensor_reduce(termsum, termr, axis=AX, op=Alu.add)
        ones = singles.tile([128, 1], f32)
        nc.vector.memset(ones, 1.0)
        ps = psum.tile([1, 1], f32, tag="big")
        nc.tensor.matmul(ps, termsum, ones, start=True, stop=True)
        res = singles.tile([1, 1], f32)
        nc.scalar.copy(res, ps)
        nc.sync.dma_start(partial.ap(), res)

    nc.compile()
    return nc


def _prep_inputs(feature, identity):
    f = np.ascontiguousarray(np.asarray(feature), dtype=np.float32)
    ids = np.asarray(identity)
    ids = ids.astype(np.int32)  # values in [0, 64); lossless from int64/int32
    assert f.shape == (N, D) and ids.shape == (N,)

    perm = np.argsort(ids, kind="stable")
    fs = f[perm]
    ids_s = ids[perm]
    maxcnt = int(np.bincount(ids_s, minlength=NID).max())
    if maxcnt <= 192:
        wide = False
    elif maxcnt <= 256:
        wide = True
    else:
        raise ValueError(f"identity group of {maxcnt} exceeds pos-window margin")

    in_maps = []
    for k in range(NCORES):
        off = (k * RPC - ROW0) % N
        fc = np.roll(fs, -off, axis=0)
        # pre-tile to [partition, tile, d] so each SBUF partition's data is
        # one contiguous DRAM run
        fc = np.ascontiguousarray(fc.reshape(N // 128, 128, D).transpose(1, 0, 2))
        in_maps.append(
            {
                "fcols": fc,
                "idcols": np.ascontiguousarray(np.roll(ids_s, -off)),
            }
        )
    return in_maps, wide


def get_nc(wide):
    key = ("nc", wide)
    if key not in _cache:
        _cache[key] = _build_nc(wide)
    return _cache[key]


def run(feature, identity, **spmd_kwargs):
    from concourse.bass_utils import run_bass_kernel_spmd

    in_maps, wide = _prep_inputs(feature, identity)
    nc = get_nc(wide)
    br = run_bass_kernel_spmd(nc, in_maps, core_ids=list(range(NCORES)), **spmd_kwargs)
    total = sum(float(r["partial"][0, 0]) for r in br.results)
    return np.asarray(np.float32(total / N)), br


def kernel(feature, identity):
    out, _ = run(feature, identity)
    return out


# revision 49
# speedup vs baseline: 1.0287x; 1.0287x over previous
"""BatchHardTripletLoss on 8 Trainium2 NeuronCores (Bass/Tile).

Math: for the n x n squared-distance matrix d2[i,j] = sq_i + sq_j - 2*f_i.f_j,
hardest positive = max_{id_j==id_i} dist, hardest negative = min_{id_j!=id_i} dist,
loss = mean(relu(margin + pos - neg)).  Both extremes commute with sqrt/+sq_i,
so each core reduces P[i,j] = delta_j - 2*G[i,j] + BIG*same[i,j] where
delta_j = sq_j - S0, then adds sq_i + S0 back in a tiny epilogue.
The BIG*same and delta_j terms ride a single K=128 one-hot matmul accumulated
on top of the Gram matmul, so no elementwise masking pass is ever needed.

Sharding: rows are sorted by identity on the host; core k owns sorted rows
[k*1024,(k+1)*1024).  Each core receives the full feature matrix rotated so its
own rows sit at local columns [256,1280) - identity groups are contiguous after
the sort, so the hardest-positive row-max only needs a 512-wide window around
the diagonal instead of a second full-matrix pass.
"""

import numpy as np

N = 8192
D = 128
NCORES = 8
RPC = N // NCORES  # rows per core
RB = RPC // 128  # row blocks per core
CHUNK = 2048  # psum chunk (4 banks)
NCHUNK = N // CHUNK
ROW0 = 256  # local column offset of a core's own rows
BIG = 4096.0
S0 = 128.0
MARGIN = 0.2
NID = 64

_cache = {}


def _build_nc(wide_window):
    from contextlib import ExitStack

    import concourse.bass as bass
    import concourse.bacc as bacc
    import concourse.mybir as mybir
    import concourse.tile as tile
    from concourse.masks import make_identity

    f32 = mybir.dt.float32
    bf16 = mybir.dt.bfloat16
    i32 = mybir.dt.int32
    AX = mybir.AxisListType.X
    Alu = mybir.AluOpType
    Act = mybir.ActivationFunctionType

    nc = bacc.Bacc(trn_type="TRN2", target_bir_lowering=False, debug=False)
    # host pre-tiles features to [p, t, d] so each partition's data is one
    # contiguous 32KB DRAM run (row-major [N, D] would DMA as a 512B scatter)
    fcols = nc.dram_tensor("fcols", [128, N // 128, D], f32, kind="ExternalInput")
    deltarow = nc.dram_tensor("deltarow", [N], f32, kind="Internal")
    idcols = nc.dram_tensor("idcols", [N], i32, kind="ExternalInput")
    partial = nc.dram_tensor("partial", [1, 1], f32, kind="ExternalOutput")

    with ExitStack() as ctx:
        tc = ctx.enter_context(tile.TileContext(nc))
        singles = ctx.enter_context(tc.tile_pool(name="singles", bufs=1))
        sqp = ctx.enter_context(tc.tile_pool(name="sqp", bufs=2))
        psum = ctx.enter_context(tc.tile_pool(name="psum", bufs=2, space="PSUM"))

        ident = singles.tile([128, 128], f32)
        make_identity(nc, ident)
        iota_i = singles.tile([64, 1], i32)
        nc.gpsimd.iota(iota_i, pattern=[[0, 1]], base=0, channel_multiplier=1)
        iota_f = singles.tile([64, 1], f32)
        nc.gpsimd.tensor_copy(iota_f, iota_i)

        # ---- DMAs up front, interleaved across the two HW DGE queues so
        # round-0 data (features + ids) lands first ----
        fnat = singles.tile([128, N // 128, D], f32)
        idb_i = singles.tile([64, N], i32)
        ic = idcols.ap()
        for r in range(4):
            eng = nc.sync if r % 2 == 0 else nc.scalar
            other = nc.scalar if r % 2 == 0 else nc.sync
            eng.dma_start(
                fnat[:, r * 16 : (r + 1) * 16, :],
                fcols.ap()[:, r * 16 : (r + 1) * 16, :],
            )
            other.dma_start(
                idb_i[:, r * CHUNK : (r + 1) * CHUNK],
                bass.AP(
                    tensor=ic.tensor, offset=r * CHUNK, ap=[[0, 64], [1, CHUNK]]
                ),
            )

        # ---- per-round: sq chain, F^T transpose, delta chunk, X chunk ----
        sqnat = singles.tile([128, N // 128], f32)  # sq of row (t*128+p) at [p, t]
        ftb = singles.tile([128, N], bf16)
        sqT = singles.tile([16, 4, 128], f32)  # [chunk-partition, round, col]
        dbc = singles.tile([64, N], f32)
        X = singles.tile([128, N], bf16)
        dr = deltarow.ap()
        # phase A: sq chain + delta broadcast + F^T transposes.  All DVE
        # reduces are emitted before any X op so the DVE stream cannot
        # head-of-line block on the id-broadcast DMA.
        sq_reduces = []
        for r in range(4):
            cols = slice(r * CHUNK, (r + 1) * CHUNK)
            # sq of this round's rows (ACT square + DVE reduce)
            sqsc = sqp.tile([128, 16, D], bf16, tag="sqsc")
            nc.scalar.activation(sqsc, fnat[:, r * 16 : (r + 1) * 16, :], Act.Square)
            sq_reduces.append(
                nc.vector.tensor_reduce(
                    sqnat[:, r * 16 : (r + 1) * 16], sqsc, axis=AX, op=Alu.add
                )
            )
            # delta chunk: transpose sq -> row layout -> DRAM -> broadcast.
            # The bounce DMAs ride the idle SP queue, not ACT's busy one.
            tq = psum.tile([128, 1024], f32, tag="big")
            nc.tensor.transpose(
                tq[0:16, 0:128], sqnat[:, r * 16 : (r + 1) * 16], ident
            )
            nc.scalar.activation(sqT[:, r, :], tq[0:16, 0:128], Act.Copy, bias=-S0)
            nc.sync.dma_start(
                bass.AP(
                    tensor=dr.tensor, offset=r * CHUNK, ap=[[128, 16], [1, 128]]
                ),
                sqT[:, r, :],
            )
            nc.sync.dma_start(
                dbc[:, cols],
                bass.AP(
                    tensor=dr.tensor, offset=r * CHUNK, ap=[[0, 64], [1, CHUNK]]
                ),
            )
            # F^T chunk in bf16 via PE transpose + ACT copyback
            for h in range(2):
                tp = psum.tile([128, 1024], f32, tag="big")
                for i in range(8):
                    nc.tensor.transpose(
                        tp[:, i * 128 : (i + 1) * 128],
                        fnat[:, r * 16 + h * 8 + i, :],
                        ident,
                    )
                nc.scalar.copy(
                    ftb[:, r * CHUNK + h * 1024 : r * CHUNK + (h + 1) * 1024], tp
                )
            if r == 0:
                ftm2 = singles.tile([128, RPC], bf16)  # -2 * own-rows slice
                nc.vector.tensor_scalar_mul(ftm2, ftb[:, ROW0 : ROW0 + RPC], -2.0)
        # phase B: X construction per chunk.  Explicit same-engine deps pin
        # every X op after the sq reduces in the DVE stream (the scheduler
        # otherwise reorders them ahead and head-of-line blocks on DMA).
        for r in range(4):
            cols = slice(r * CHUNK, (r + 1) * CHUNK)
            # X rhs rows 0:64 -> onehot(id_j)*delta_j; rows 64:128 -> 64*onehot
            i1 = nc.vector.scalar_tensor_tensor(
                X[0:64, cols],
                idb_i[:, cols],
                iota_f,
                dbc[:, cols],
                op0=Alu.is_equal,
                op1=Alu.mult,
            )
            i2 = nc.vector.tensor_scalar(
                X[64:128, cols],
                idb_i[:, cols],
                iota_f,
                64.0,
                op0=Alu.is_equal,
                op1=Alu.mult,
            )
            for xi in (i1, i2):
                tile.add_dep_helper(
                    xi.ins, sq_reduces[-1].ins, sync=False, reason="dve order"
                )
            if r == 0:
                # lhsT for extras: ones on top, 64*onehot(id_m) below
                XL = singles.tile([128, RPC], bf16)
                nc.vector.memset(XL[0:64, :], 1.0)
                nc.vector.tensor_copy(XL[64:128, :], X[64:128, ROW0 : ROW0 + RPC])

        # per-row-block epilogue biases: sq_m + S0 (and -BIG for the pos side)
        biasP = singles.tile([128, RB], f32)
        nc.vector.tensor_scalar_add(biasP, sqnat[:, 2 : 2 + RB], S0 - BIG)
        biasN = singles.tile([128, RB], f32)
        nc.vector.tensor_scalar_add(biasN, sqnat[:, 2 : 2 + RB], S0)

        # ---- main loop ----
        # MCH=1024 psum tiles x 4 bufs keep PE streaming (2 chunks in
        # flight ahead of the DVE reduces) so the HAM clock stays warm.
        MCH = 1024
        NMC = N // MCH
        negacc = singles.tile([128, RB, NMC], f32)
        posacc = singles.tile([128, RB, 2], f32)
        nc.vector.memset(posacc, -1e9)
        MMF = 512  # psum-bank-limited moving-operand width
        # c outer / rb inner: chunk-c work only needs X columns of chunk c,
        # so the main loop starts as soon as the first X chunk is ready
        for c in range(NMC):
            for rb in range(RB):
                if wide_window:
                    wlo, whi = 0, 2048
                else:
                    wlo, whi = rb * 128 + 64, rb * 128 + 576
                P = psum.tile([128, MCH], f32, tag="big")
                # grouped by stationary operand so LDWEIGHTS amortizes
                for s in range(MCH // MMF):
                    col = c * MCH + s * MMF
                    nc.tensor.matmul(
                        P[:, s * MMF : (s + 1) * MMF],
                        ftm2[:, rb * 128 : (rb + 1) * 128],
                        ftb[:, col : col + MMF],
                        start=True,
                        stop=False,
                    )
                for s in range(MCH // MMF):
                    col = c * MCH + s * MMF
                    nc.tensor.matmul(
                        P[:, s * MMF : (s + 1) * MMF],
                        XL[:, rb * 128 : (rb + 1) * 128],
                        X[:, col : col + MMF],
                        start=False,
                        stop=True,
                    )
                nc.vector.tensor_reduce(
                    negacc[:, rb, c : c + 1], P, axis=AX, op=Alu.min
                )
                lo = max(wlo, c * MCH) - c * MCH
                hi = min(whi, (c + 1) * MCH) - c * MCH
                if lo < hi:
                    nc.vector.tensor_reduce(
                        posacc[:, rb, c % 2 : c % 2 + 1],
                        P[:, lo:hi],
                        axis=AX,
                        op=Alu.max,
                    )

        # ---- epilogue: sqrt both sides, relu(margin + pos - neg), sum ----
        negmin = singles.tile([128, RB], f32)
        nc.vector.tensor_reduce(negmin, negacc, axis=AX, op=Alu.min)
        posmax = singles.tile([128, RB], f32)
        nc.vector.tensor_reduce(posmax, posacc, axis=AX, op=Alu.max)
        posd2 = singles.tile([128, RB], f32)
        nc.vector.tensor_tensor(posd2, posmax, biasP, op=Alu.add)
        negd2 = singles.tile([128, RB], f32)
        nc.vector.tensor_tensor(negd2, negmin, biasN, op=Alu.add)
        posd = singles.tile([128, RB], f32)
        nc.scalar.activation(posd, posd2, Act.Sqrt)
        negd = singles.tile([128, RB], f32)
        nc.scalar.activation(negd, negd2, Act.Sqrt)
        term = singles.tile([128, RB], f32)
        nc.vector.scalar_tensor_tensor(
            term, posd, MARGIN, negd, op0=Alu.add, op1=Alu.subtract
        )
        termr = singles.tile([128, RB], f32)
        nc.vector.tensor_scalar_max(termr, term, 0.0)
        termsum = singles.tile([128, 1], f32)
        nc.vector.tensor_reduce(termsum, termr, axis=AX, op=Alu.add)
        ones = singles.tile([128, 1], f32)
        nc.vector.memset(ones, 1.0)
        ps = psum.tile([1, 1], f32, tag="big")
        nc.tensor.matmul(ps, termsum, ones, start=True, stop=True)
        res = singles.tile([1, 1], f32)
        nc.scalar.copy(res, ps)
        nc.sync.dma_start(partial.ap(), res)

    nc.compile()
    return nc


def _prep_inputs(feature, identity):
    f = np.ascontiguousarray(np.asarray(feature), dtype=np.float32)
    ids = np.asarray(identity)
    ids = ids.astype(np.int32)  # values in [0, 64); lossless from int64/int32
    assert f.shape == (N, D) and ids.shape == (N,)

    perm = np.argsort(ids, kind="stable")
    fs = f[perm]
    ids_s = ids[perm]
    maxcnt = int(np.bincount(ids_s, minlength=NID).max())
    if maxcnt <= 192:
        wide = False
    elif maxcnt <= 256:
        wide = True
    else:
        raise ValueError(f"identity group of {maxcnt} exceeds pos-window margin")

    in_maps = []
    for k in range(NCORES):
        off = (k * RPC - ROW0) % N
        fc = np.roll(fs, -off, axis=0)
        # pre-tile to [partition, tile, d] so each SBUF partition's data is
        # one contiguous DRAM run
        fc = np.ascontiguousarray(fc.reshape(N // 128, 128, D).transpose(1, 0, 2))
        in_maps.append(
            {
                "fcols": fc,
                "idcols": np.ascontiguousarray(np.roll(ids_s, -off)),
            }
        )
    return in_maps, wide


def get_nc(wide):
    key = ("nc", wide)
    if key not in _cache:
        _cache[key] = _build_nc(wide)
    return _cache[key]


def run(feature, identity, **spmd_kwargs):
    from concourse.bass_utils import run_bass_kernel_spmd

    in_maps, wide = _prep_inputs(feature, identity)
    nc = get_nc(wide)
    br = run_bass_kernel_spmd(nc, in_maps, core_ids=list(range(NCORES)), **spmd_kwargs)
    total = sum(float(r["partial"][0, 0]) for r in br.results)
    return np.asarray(np.float32(total / N)), br


def kernel(feature, identity):
    out, _ = run(feature, identity)
    return out
